# revision 9
# baseline (speedup 1.0000x reference)
"""CvT attention block kernel for Trainium2 (8 NeuronCores, batch-parallel).

Problem: B=32 samples of x (C=128, 32x32 lattice -> N=1024 tokens),
8 heads x 64 dk attention with a relative-position bias expanded from
R (8, 32, 32), residual output.  Sharding: 4 samples per core.

Per-sample math (reference):
    xn  = x / sqrt(5);  xf = xn.reshape(C, N)
    Q/K/V = W{q,k,v} @ xf               (512, N)
    dot = Q_h^T K_h + B_h               (N, N) per head
    alpha = softmax(dot / 8, axis=-1)
    att = alpha @ V_h^T                 -> (512, N)
    out = W0 @ att + x

Kernel strategy (transposed scores, 2-byte dtypes on the PE):
    S^T[j, i] = sum_d K[d,j] Q[d,i]    (keys j on partitions)
    alpha^T = exp(S^T / 8) * expB^T    (expB = exp(B^T/8), block-circulant
                                        table with per-partition-group shifts
                                        baked in so each (h, jb) bias tile is
                                        a contiguous fp16 slice)
    att[d, i] = sum_j VT[j, d] alpha^T[j, i]  accumulated over key blocks,
    with a ones-column in VT producing the softmax denominator at row 64;
    normalization via fast-reciprocal + K=1 broadcast matmuls.
"""

import math

import numpy as np

import concourse.bass as bass
import concourse.bacc as bacc
import concourse.mybir as mybir
import concourse.tile as tile
from concourse.bass_utils import run_bass_kernel_spmd

B, C, L, HEADS, DK = 32, 128, 32, 8, 64
N = L * L  # 1024 tokens
NCORES = 8
BPC = B // NCORES  # samples per core
NLAYER = 4
INV_LAYER = 1.0 / math.sqrt(NLAYER + 1)
SM_SCALE = 1.0 / math.sqrt(DK)  # 0.125

F32 = mybir.dt.float32
F16 = mybir.dt.float16

# expB table geometry: width 2048 per head, slice offset per key-block jb
EXPB_W = 2048


def _expb_offset(jb: int) -> int:
    return 1024 - 128 * jb


def make_expb_table(R: np.ndarray) -> np.ndarray:
    """Build the multiplicative-bias table, fp16, shape (128, HEADS*2048).

    For the mult  alpha = exp(S^T/8) * expB_slice :
      score tile for (h, jb) is (128 keys, 1024 queries) with key partition
      p = g*32 + yk  (g in 0..3 selects xj = 4*jb + g), query free index
      f = xi*32 + yi.
      required value = exp(0.125 * R[h, (xi - xj) % 32, (yi - yk) % 32]).

    table[h] is built so that tile slice = table[:, h*2048 + F(jb) : +1024]
    with F(jb) = 1024 - 128*jb, by storing, for partition group g, the
    doubly-expanded circulant shifted right by 32*g.
    """
    R = np.asarray(R, np.float64)
    ys = np.arange(L)
    dyy = (ys[:, None] - ys[None, :]) % L  # dyy[yi, yk] = (yi-yk)%L
    table = np.zeros((128, HEADS * EXPB_W), np.float64)
    for h in range(HEADS):
        e = np.exp(SM_SCALE * R[h])  # (32, 32) indexed [dx, dy]
        Cu = e[:, dyy.T]  # (dx, yk, yi) = e[dx, (yi-yk)%32]
        Cu = np.concatenate([Cu, Cu], axis=0)  # (64, yk, yi), u%32 semantics
        D = Cu.transpose(1, 0, 2).reshape(L, 64 * L)  # D[yk, u*32+yi]
        for g in range(4):
            sh = 32 * g
            table[g * 32:(g + 1) * 32, h * EXPB_W + sh:(h + 1) * EXPB_W] = \
                D[:, :EXPB_W - sh]
    return table.astype(np.float16)


def build_nc(num_samples: int = BPC, gp_mult_every: int = 3,
             use_seq_codegen: bool = False) -> bass.Bass:
    """Emit the per-core Bass/Tile kernel for `num_samples` samples."""
    nc = bacc.Bacc(use_seq_codegen=use_seq_codegen)

    x_in = nc.dram_tensor("x_in", (num_samples, C, N), F32, kind="ExternalInput")
    wqT_d = nc.dram_tensor("wqT", (C, 512), F16, kind="ExternalInput")
    wkT_d = nc.dram_tensor("wkT", (C, 512), F16, kind="ExternalInput")
    wvT_d = nc.dram_tensor("wvT", (C, 512), F16, kind="ExternalInput")
    w0T_d = nc.dram_tensor("w0T", (C, 512), F16, kind="ExternalInput")
    expb_d = nc.dram_tensor("expB", (C, HEADS * EXPB_W), F16, kind="ExternalInput")
    x_out = nc.dram_tensor("x_out", (num_samples, C, N), F32, kind="ExternalOutput")

    with tile.TileContext(nc) as tc:
        with (
            tc.tile_pool(name="const", bufs=1) as constp,
            tc.tile_pool(name="xf", bufs=2) as xfp,
            tc.tile_pool(name="xb", bufs=2) as xbp,
            tc.tile_pool(name="qk", bufs=2) as qkp,
            tc.tile_pool(name="vt", bufs=2) as vtp,
            tc.tile_pool(name="alpha", bufs=4) as alphap,
            tc.tile_pool(name="attsb", bufs=6) as attsbp,
            tc.tile_pool(name="recip", bufs=2) as recipp,
            tc.tile_pool(name="rbc", bufs=2) as rbcp,
            tc.tile_pool(name="outsb", bufs=2) as outp,
            tc.tile_pool(name="rcd", bufs=2, space="DRAM") as rcdp,
            tc.tile_pool(name="ps2", bufs=2, space="PSUM") as ps2,  # 2-bank slots
            tc.tile_pool(name="attps", bufs=1, space="PSUM") as attps,  # 4-bank slot
        ):
            # ---- constants ----
            wq_sb = constp.tile([C, 512], F16, tag="wq")
            wk_sb = constp.tile([C, 512], F16, tag="wk")
            wv_sb = constp.tile([C, 512], F16, tag="wv")
            w0_sb = constp.tile([C, 512], F16, tag="w0")
            expb_sb = constp.tile([C, HEADS * EXPB_W], F16, tag="expb")
            nc.sync.dma_start(wq_sb[:], wqT_d[:])
            nc.sync.dma_start(wk_sb[:], wkT_d[:])
            nc.sync.dma_start(wv_sb[:], wvT_d[:])
            nc.sync.dma_start(w0_sb[:], w0T_d[:])
            nc.sync.dma_start(expb_sb[:], expb_d[:])

            for b in range(num_samples):
                # ---- load + cast ----
                xf = xfp.tile([C, N], F32)
                nc.sync.dma_start(xf[:], x_in[b])
                xb = xbp.tile([C, N], F16)
                nc.gpsimd.tensor_copy(xb[:], xf[:])

                # ---- Q, K projections: (hd=128 rows per tile t, N) ----
                q_sb = qkp.tile([C, 4 * N], F16, tag="q")
                k_sb = qkp.tile([C, 4 * N], F16, tag="k")
                for t in range(4):
                    for w_sb, dst in ((wq_sb, q_sb), (wk_sb, k_sb)):
                        ps = ps2.tile([C, N], F32, tag="ps2")
                        for ih in range(2):
                            nc.tensor.matmul(
                                ps[:, ih * 512:(ih + 1) * 512],
                                w_sb[:, t * 128:(t + 1) * 128],
                                xb[:, ih * 512:(ih + 1) * 512],
                                start=True, stop=True,
                            )
                        nc.vector.tensor_copy(dst[:, t * N:(t + 1) * N], ps[:])

                # ---- V^T: per key-block jb, (128 tokens, 8h*65) ----
                vt_sb = vtp.tile([C, 8 * 65 * 8], F16)  # (128, 4160)
                for jb in range(8):
                    ps = ps2.tile([C, N], F32, tag="ps2")
                    nc.tensor.matmul(
                        ps[:, 0:512], xb[:, jb * 128:(jb + 1) * 128], wv_sb[:],
                        start=True, stop=True,
                    )
                    seg = vt_sb[:, jb * 520:(jb + 1) * 520]
                    seg3 = seg.rearrange("p (h d) -> p h d", d=65)
                    nc.vector.tensor_copy(
                        seg3[:, :, 0:64],
                        ps[:, 0:512].rearrange("p (h d) -> p h d", d=64),
                    )
                    nc.gpsimd.memset(seg3[:, :, 64:65], 1.0)

                # ---- attention per head pair hp ----
                att_sb = []
                for hp in range(4):
                    a_sb = attsbp.tile([C, N], F16)
                    att_sb.append(a_sb)
                    att_ps = attps.tile([65, 4 * 512], F32, tag="attps")
                    for jb in range(8):
                        for p in range(2):  # head parity (row-packed pairs)
                            h = 2 * hp + p
                            sl = slice(p * 64, (p + 1) * 64)
                            s_ps = ps2.tile([C, N], F32, tag="ps2")
                            for ih in range(2):
                                nc.tensor.matmul(
                                    s_ps[:, ih * 512:(ih + 1) * 512],
                                    k_sb[sl, hp * N + jb * 128: hp * N + (jb + 1) * 128],
                                    q_sb[sl, hp * N + ih * 512: hp * N + (ih + 1) * 512],
                                    start=True, stop=True,
                                )
                            a0 = alphap.tile([C, N], F16, tag="a0")
                            nc.scalar.activation(
                                a0[:], s_ps[:],
                                mybir.ActivationFunctionType.Exp,
                                scale=SM_SCALE,
                            )
                            al = alphap.tile([C, N], F16, tag="al")
                            eb = expb_sb[:, h * EXPB_W + _expb_offset(jb):
                                         h * EXPB_W + _expb_offset(jb) + N]
                            eng = (nc.gpsimd if (gp_mult_every and
                                                 jb % gp_mult_every == gp_mult_every - 1)
                                   else nc.vector)
                            eng.tensor_mul(al[:], a0[:], eb)
                            # ---- A@V^T accumulation (+ ones col -> denom) ----
                            for ih in range(2):
                                nc.tensor.matmul(
                                    att_ps[0:65, (p * 2 + ih) * 512:(p * 2 + ih + 1) * 512],
                                    vt_sb[:, jb * 520 + h * 65: jb * 520 + h * 65 + 65],
                                    al[:, ih * 512:(ih + 1) * 512],
                                    start=(jb == 0), stop=(jb == 7),
                                )
                    # ---- normalize: recip of denom row, DMA broadcast, mult ----
                    # (custom-DVE ops lose the input partition offset on HW, so
                    # first move row 64 to a base-0 SBUF tile with a plain copy)
                    den = recipp.tile([1, 4 * 512], F32, tag="den")
                    nc.vector.tensor_copy(den[:], att_ps[64:65, :])
                    rc = recipp.tile([1, 4 * 512], F32, tag="rc")
                    nc.vector.reciprocal_approx_fast(rc[:], den[:])
                    rcd = rcdp.tile([1, 4 * 512], F32)
                    nc.sync.dma_start(rcd[:], rc[:])
                    rbc = rbcp.tile([64, 4 * 512], F32)
                    nc.sync.dma_start(
                        rbc[:],
                        bass.AP(tensor=rcd.tensor, offset=rcd.offset,
                                ap=[[0, 64], [1, 4 * 512]]),
                    )
                    for p in range(2):
                        nc.vector.tensor_mul(
                            a_sb[p * 64:(p + 1) * 64, :],
                            att_ps[0:64, p * 1024:(p + 1) * 1024],
                            rbc[0:64, p * 1024:(p + 1) * 1024],
                        )

                # ---- output projection + residual ----
                out_sb = outp.tile([C, N], F32)
                for ih in range(2):
                    po = ps2.tile([C, 512], F32, tag="ps2")
                    for hp in range(4):
                        nc.tensor.matmul(
                            po[:], w0_sb[:, hp * 128:(hp + 1) * 128],
                            att_sb[hp][:, ih * 512:(ih + 1) * 512],
                            start=(hp == 0), stop=(hp == 3),
                        )
                    nc.vector.tensor_add(
                        out_sb[:, ih * 512:(ih + 1) * 512], po[:],
                        xf[:, ih * 512:(ih + 1) * 512],
                    )
                nc.sync.dma_start(x_out[b], out_sb[:])

    nc.finalize()
    return nc


def prep_weights(Wq, Wk, Wv, W0):
    """Host-side: transpose, fold in the 1/sqrt(NLAYER+1) prescale, cast."""
    wqT = (np.asarray(Wq, np.float64).T * INV_LAYER).astype(np.float16)
    wkT = (np.asarray(Wk, np.float64).T * INV_LAYER).astype(np.float16)
    wvT = (np.asarray(Wv, np.float64).T * INV_LAYER).astype(np.float16)
    # w0T[p, hp*128 + c] = W0[c, hp*128 + p]
    w0 = np.asarray(W0, np.float64)
    w0T = np.concatenate([w0.T[k * 128:(k + 1) * 128, :] for k in range(4)],
                         axis=1).astype(np.float16)
    return wqT, wkT, wvT, w0T


_NC_CACHE: dict = {}


def kernel(x, Wq, Wk, Wv, R, W0):
    x = np.ascontiguousarray(np.asarray(x, np.float32))
    wqT, wkT, wvT, w0T = prep_weights(Wq, Wk, Wv, W0)
    expb = np.ascontiguousarray(make_expb_table(np.asarray(R, np.float32)))

    if "nc" not in _NC_CACHE:
        _NC_CACHE["nc"] = build_nc(BPC)
    nc = _NC_CACHE["nc"]

    xs = x.reshape(B, C, N)
    in_maps = []
    for c in range(NCORES):
        in_maps.append({
            "x_in": np.ascontiguousarray(xs[c * BPC:(c + 1) * BPC]),
            "wqT": wqT, "wkT": wkT, "wvT": wvT, "w0T": w0T,
            "expB": expb,
        })
    res = run_bass_kernel_spmd(nc, in_maps, core_ids=list(range(NCORES)))
    out = np.concatenate([r["x_out"] for r in res.results], axis=0)
    return out.reshape(B, C, L, L)
